# revision 1
# baseline (speedup 1.0000x reference)
"""GPT2-style fused attention (DecisionTransformer) on 8 Trainium2 NeuronCores.

Sharding: tensor-parallel over the 16 heads (2 heads per core, both batch
elements on every core).  Each core:
  - loads the full hidden_states [4096, 1024],
  - computes Q/K/V for its 2 heads (transposed layout via PE transposes),
  - causal attention for its 4 (batch, head) pairs: scores^T = K @ Q^T,
    exp (no max subtraction -- logits are small and bounded), ones-column
    appended to V gives the softmax denominator for free in the A@V matmul,
  - row-parallel output projection with its 128 rows of c_proj_w,
  - writes a full-shape partial output [4096, 1024].
Host gathers with a sum over the 8 partials (the row-parallel all-reduce)
and adds c_proj_b.

Matmuls run in float32r (full-rate fp32 streaming, ~tf32-like rounding);
measured output error vs the fp32 reference is ~2e-4 relative to absmax.
"""

import sys

for _p in ("/opt/trn_rl_repo",):
    if _p not in sys.path:
        sys.path.insert(0, _p)

import numpy as np

import concourse.bass as bass
import concourse.mybir as mybir
import concourse.tile as tile
from concourse import bacc
from concourse.bass_utils import run_bass_kernel_spmd
from concourse.masks import make_identity

P = 128
B, S, D, H, HD = 2, 2048, 1024, 16, 64
T = B * S              # 4096 tokens
FQKV = 3 * P           # 384 per-core qkv features (q128 | k128 | v128)
KO = D // P            # 8 contraction chunks
TCH = 512              # token chunk for qkv phase
NTCH = T // TCH        # 8
QC = 512               # query chunk in attention
NQC = S // QC          # 4
NKB = S // P           # 16 key blocks per sequence
SCALE = 1.0 / float(HD) ** 0.5
N_CORES = 8
HPC = H // N_CORES     # 2 heads per core

f32 = mybir.dt.float32
f32r = mybir.dt.float32r
MM_DT = f32r


def _emit_body(nc, tc, pools, consts, it, phases='full'):
    (xin_pool, xt_pool, qkvt_pool, vaug_pool, pt_pool, atn_pool, out_pool,
     small_pool, ps_mm, ps_s, ps_o) = pools
    (wqkv_sb, wp_sb, bqkv_sb, ident_f32, identr, ident2, mask128,
     ones1, x_d, out_d, xi_pre) = consts

    # per-batch K^T / V^T / padded-Q^T tiles so batch 1's projection can
    # overlap batch 0's attention (no shared-tile false dependencies)
    ktb = [qkvt_pool.tile([P, S], MM_DT, tag=f"kt{b}", name=f"kt{b}")
           for b in range(B)]
    vtb = [qkvt_pool.tile([P, S], MM_DT, tag=f"vt{b}", name=f"vt{b}")
           for b in range(B)]
    # Q^T per (batch, local head), zero-padded to 128 contraction rows: the
    # other head's 64 rows stay zero so a full-128-partition matmul against
    # the stacked K^T contracts exactly (sub-128 matmuls run at half rate).
    qpad = [
        [qkvt_pool.tile([P, S], MM_DT, tag=f"qp{b}{h}", name=f"qp{b}{h}")
         for h in range(HPC)]
        for b in range(B)
    ]
    if it == 0:
        for b in range(B):
            nc.vector.memset(qpad[b][0][HD:, :].bitcast(f32), 0.0)
            nc.vector.memset(qpad[b][1][:HD, :].bitcast(f32), 0.0)
    vaug = [
        vaug_pool.tile([P, NKB, P], MM_DT, tag=f"vaug{p}", name=f"vaug{p}")
        for p in range(B * HPC)
    ]
    atn = [
        [
            atn_pool.tile([P, QC], MM_DT, tag=f"atn{b}_{q}", name=f"atn{b}_{q}")
            for q in range(NQC)
        ]
        for b in range(B)
    ]

    # ---- phases 1-3 per batch: X^T, QKV projection, V_aug ----
    for b in range(B):
        for i in range(S // TCH):
            gi = b * (S // TCH) + i
            xt = xt_pool.tile([P, KO, TCH], MM_DT, tag="xt", name="xt")
            if it == 0 and gi == 0:
                xins = xi_pre
            else:
                xins = []
                for j in range(TCH // P):
                    xi = xin_pool.tile([P, D], f32, tag="xi", name="xi")
                    nc.sync.dma_start(
                        xi[:], x_d[gi * TCH + j * P : gi * TCH + (j + 1) * P, :]
                    )
                    xins.append(xi)
            # 4 PE transposes share one PSUM bank -> single wide eviction
            for ko in range(KO):
                ps = ps_mm.tile([P, TCH], f32, tag="mm", name="psmm")
                for j in range(TCH // P):
                    nc.tensor.transpose(
                        ps[:, j * P : (j + 1) * P],
                        xins[j][:, ko * P : (ko + 1) * P],
                        ident_f32[:],
                    )
                nc.scalar.copy(xt[:, ko, :], ps[:])
            for fc in range(3):
                ps = ps_mm.tile([P, TCH], f32, tag="mm", name="psmm")
                for ko in range(KO):
                    nc.tensor.matmul(
                        ps[:],
                        wqkv_sb[:, ko, fc * P : (fc + 1) * P],
                        xt[:, ko, :],
                        start=(ko == 0),
                        stop=(ko == KO - 1),
                    )
                # evict + per-partition bias add on DVE
                cs = slice(i * TCH, (i + 1) * TCH)
                if fc == 0:
                    nc.vector.tensor_scalar(
                        qpad[b][0][:HD, cs], ps[:HD],
                        bqkv_sb[:HD, fc : fc + 1], None, mybir.AluOpType.add,
                    )
                    nc.vector.tensor_scalar(
                        qpad[b][1][HD:, cs], ps[HD:],
                        bqkv_sb[HD:, fc : fc + 1], None, mybir.AluOpType.add,
                    )
                else:
                    dst = ktb[b] if fc == 1 else vtb[b]
                    nc.vector.tensor_scalar(
                        dst[:, cs], ps[:],
                        bqkv_sb[:, fc : fc + 1], None, mybir.AluOpType.add,
                    )
        # V_aug for this batch (V back to natural layout + ones column)
        for hl in range(HPC):
            p = b * HPC + hl
            vt = vtb[b][hl * HD : (hl + 1) * HD, :]
            if it == 0:
                nc.vector.memset(vaug[p][:, :, HD : HD + 1].bitcast(f32), 1.0)
                nc.vector.memset(vaug[p][:, :, HD + 1 :].bitcast(f32), 0.0)
            for kb in range(0, NKB, 2):
                ps = ps_mm.tile([P, TCH], f32, tag="mm", name="psmm")
                for u in range(2):
                    nc.tensor.transpose(
                        ps[:, u * HD : (u + 1) * HD].bitcast(f32r),
                        vt[:, (kb + u) * P : (kb + u + 1) * P],
                        ident2[hl * HD : (hl + 1) * HD, :],
                    )
                nc.vector.tensor_copy(
                    vaug[p][:, kb : kb + 2, :HD],
                    ps[:, : 2 * HD].rearrange("p (u h) -> p u h", u=2),
                )

    if phases == 'a':
        return
    # ---- phase 4+5: attention + output projection ----
    for b in range(B):
        for qc in range(NQC):
            for hl in range(HPC):
                p = b * HPC + hl
                rhs_q = qpad[b][hl][:, qc * QC : (qc + 1) * QC]
                po = ps_o.tile([P, QC], f32, tag="po", name="pso")
                nkb = (qc + 1) * (QC // P)
                for kb in range(nkb):
                    j = kb - qc * (QC // P)
                    lo = j * P if j > 0 else 0
                    ps = ps_s.tile([P, QC], f32, tag="s", name="pss")
                    nc.tensor.matmul(
                        ps[:, lo:],
                        ktb[b][:, kb * P : (kb + 1) * P],
                        rhs_q[:, lo:],
                        start=True,
                        stop=True,
                    )
                    pt = pt_pool.tile([P, QC], MM_DT, tag="pt", name="pt")
                    if j < 0:
                        nc.scalar.activation(
                            pt[:],
                            ps[:],
                            mybir.ActivationFunctionType.Exp,
                            scale=SCALE,
                        )
                        nc.tensor.matmul(
                            po[:],
                            vaug[p][:, kb, :],
                            pt[:],
                            start=(kb == 0),
                            stop=False,
                        )
                    else:
                        # diagonal block: only cols >= j*128 are live; the
                        # A@V matmul covers just that column range, so the
                        # masked region needs no zeroing at all.
                        nc.scalar.activation(
                            pt[:, j * P :],
                            ps[:, j * P :],
                            mybir.ActivationFunctionType.Exp,
                            scale=SCALE,
                        )
                        nc.vector.tensor_tensor(
                            pt[:, j * P : (j + 1) * P],
                            pt[:, j * P : (j + 1) * P],
                            mask128[:],
                            mybir.AluOpType.mult,
                        )
                        nc.tensor.matmul(
                            po[:, j * P :],
                            vaug[p][:, kb, :],
                            pt[:, j * P :],
                            start=(kb == 0),
                            stop=(kb == nkb - 1),
                        )
                # normalize: A^T = O^T_u * (1/denom), denom = po[64].
                # Broadcast denom across 64 partitions FIRST (rank-1 PE
                # matmul), then reciprocal on 64 lanes -- a [1,512]
                # single-lane reciprocal measures ~3.4us on HW.
                den = small_pool.tile([1, QC], MM_DT, tag="rec", name="rec")
                nc.vector.tensor_copy(den[:], po[HD : HD + 1, :])
                rbc = ps_mm.tile([P, TCH], f32, tag="mm", name="psmm")[:HD, :QC]
                nc.tensor.matmul(
                    rbc[:],
                    ones1[:, :HD],
                    den[:],
                    start=True,
                    stop=True,
                )
                rbs = small_pool.tile([HD, QC], f32, tag="rbs", name="rbs")
                # ~51 ULP approx (plenty for softmax denominators), ~5x
                # faster than the exact DVE reciprocal
                nc.vector.reciprocal_approx_fast(out=rbs[:], in_=rbc[:])
                nc.vector.tensor_tensor(
                    atn[b][qc][hl * HD : (hl + 1) * HD, :],
                    po[:HD, :],
                    rbs[:],
                    mybir.AluOpType.mult,
                )
            # output projection for this (b, qc)
            for qb in range(QC // P):
                for nck in range(2):
                    pp = ps_mm.tile([P, TCH], f32, tag="mm", name="psmm")
                    nc.tensor.matmul(
                        pp[:, :512],
                        atn[b][qc][:, qb * P : (qb + 1) * P],
                        wp_sb[:, nck * 512 : (nck + 1) * 512],
                        start=True,
                        stop=True,
                    )
                    ot = out_pool.tile([P, 512], f32, tag="ot", name="ot")
                    nc.vector.tensor_copy(ot[:], pp[:, :512])
                    row = b * S + qc * QC + qb * P
                    nc.sync.dma_start(
                        out_d[row : row + P, nck * 512 : (nck + 1) * 512],
                        ot[:],
                    )


def _build_program(iters=1, phases='full'):
    nc = bacc.Bacc(None, target_bir_lowering=False)

    x_d = nc.dram_tensor("x", [T, D], f32, kind="ExternalInput")
    wqkv_d = nc.dram_tensor("w_qkv", [D, FQKV], f32, kind="ExternalInput")
    bqkv_d = nc.dram_tensor("b_qkv", [FQKV], f32, kind="ExternalInput")
    wp_d = nc.dram_tensor("w_proj", [P, D], f32, kind="ExternalInput")
    out_d = nc.dram_tensor("out", [T, D], f32, kind="ExternalOutput")

    with tile.TileContext(nc) as tc:
        with (
            tc.tile_pool(name="const", bufs=1) as const,
            tc.tile_pool(name="xin", bufs=4) as xin_pool,
            tc.tile_pool(name="xt", bufs=2) as xt_pool,
            tc.tile_pool(name="qkvt", bufs=1) as qkvt_pool,
            tc.tile_pool(name="vaug", bufs=1) as vaug_pool,
            tc.tile_pool(name="pt", bufs=5) as pt_pool,
            tc.tile_pool(name="atn", bufs=1) as atn_pool,
            tc.tile_pool(name="outp", bufs=3) as out_pool,
            tc.tile_pool(name="small", bufs=3) as small_pool,
            tc.tile_pool(name="ps_mm", bufs=3, space="PSUM") as ps_mm,
            tc.tile_pool(name="ps_s", bufs=3, space="PSUM") as ps_s,
            tc.tile_pool(name="ps_o", bufs=2, space="PSUM") as ps_o,
        ):
            # ---- constants ----
            # prefetch the first token chunk before the (large) weight DMAs
            # so the transpose pipeline starts immediately
            xi_pre = []
            for j in range(TCH // P):
                xi = xin_pool.tile([P, D], f32, tag="xi", name="xi")
                nc.sync.dma_start(xi[:], x_d[j * P : (j + 1) * P, :])
                xi_pre.append(xi)
            # weights: gpsimd "casting" DMA fp32 -> f32r (bit-identical move;
            # satisfies the BIR fp32r-producer rule)
            wqkv_sb = const.tile([P, KO, FQKV], MM_DT)
            wq_stage = xt_pool.tile([P, KO, FQKV], f32, tag="xt", name="xt")
            nc.sync.dma_start(
                wq_stage[:], wqkv_d.rearrange("(ko p) f -> p ko f", p=P)
            )
            nc.vector.tensor_copy(wqkv_sb[:], wq_stage[:])
            wp_sb = const.tile([P, D], MM_DT)
            wp_stage = xin_pool.tile([P, D], f32, tag="xi", name="xi")
            nc.sync.dma_start(wp_stage[:], wp_d[:])
            nc.vector.tensor_copy(wp_sb[:], wp_stage[:])
            bqkv_sb = const.tile([P, 3], f32)
            nc.sync.dma_start(bqkv_sb[:], bqkv_d.rearrange("(c p) -> p c", p=P))
            ident_f32 = const.tile([P, P], f32)
            make_identity(nc, ident_f32[:])
            identr = const.tile([P, P], MM_DT)
            nc.vector.tensor_copy(identr[:], ident_f32[:])
            # ident2[r, c] = 1 iff r == c or r == c + 64 (c < 64): slices
            # [:64] / [64:] are 64x64 identities at partition base 0 / 64,
            # for transposing the per-head V^T chunks (lhsT and rhs of a
            # matmul must share the same base partition).
            for w in range(56):
                ps_warm = ps_s.tile([P, QC], f32, tag="s", name="pss")
                nc.tensor.matmul(
                    ps_warm[:, :P], ident_f32[:], ident_f32[:],
                    start=True, stop=True,
                )
            ident2_f32 = const.tile([P, HD], f32)
            nc.gpsimd.memset(ident2_f32[:], 0.0)
            for base in (0, -HD):
                nc.gpsimd.affine_select(
                    out=ident2_f32[:],
                    in_=ident2_f32[:],
                    compare_op=mybir.AluOpType.not_equal,
                    fill=1.0,
                    base=base,
                    pattern=[[-1, HD]],
                    channel_multiplier=1,
                )
            ident2 = const.tile([P, HD], MM_DT)
            nc.vector.tensor_copy(ident2[:], ident2_f32[:])
            ones1 = const.tile([1, P], MM_DT)
            nc.vector.memset(ones1[:].bitcast(f32), 1.0)
            # mask128[k, q] = 1.0 if k <= q else 0.0
            mask128 = const.tile([P, P], f32)
            nc.gpsimd.memset(mask128[:], 1.0)
            nc.gpsimd.affine_select(
                out=mask128[:],
                in_=mask128[:],
                compare_op=mybir.AluOpType.is_ge,
                fill=0.0,
                base=0,
                pattern=[[1, P]],
                channel_multiplier=-1,
            )

            pools = (xin_pool, xt_pool, qkvt_pool, vaug_pool, pt_pool,
                     atn_pool, out_pool, small_pool, ps_mm, ps_s, ps_o)
            consts = (wqkv_sb, wp_sb, bqkv_sb, ident_f32, identr, ident2,
                      mask128, ones1, x_d, out_d, xi_pre)
            for it in range(iters):
                _emit_body(nc, tc, pools, consts, it, phases)

    nc.compile()
    return nc


_CACHE = {}


def get_program(iters=1, phases='full'):
    key = (iters, phases)
    if key not in _CACHE:
        _CACHE[key] = _build_program(iters, phases)
    return _CACHE[key]


def make_in_maps(hidden_states, c_attn_w, c_attn_b, c_proj_w):
    x = np.ascontiguousarray(
        np.asarray(hidden_states, dtype=np.float32).reshape(T, D)
    )
    wa = np.asarray(c_attn_w, dtype=np.float32)
    ba = np.asarray(c_attn_b, dtype=np.float32)
    wp = np.asarray(c_proj_w, dtype=np.float32)
    in_maps = []
    for c in range(N_CORES):
        lo, hi = c * P, (c + 1) * P
        w_qkv = np.ascontiguousarray(
            np.concatenate(
                [wa[:, lo:hi], wa[:, D + lo : D + hi], wa[:, 2 * D + lo : 2 * D + hi]],
                axis=1,
            )
        )
        b_qkv = np.ascontiguousarray(
            np.concatenate([ba[lo:hi], ba[D + lo : D + hi], ba[2 * D + lo : 2 * D + hi]])
        )
        w_proj = np.ascontiguousarray(wp[lo:hi, :])
        in_maps.append({"x": x, "w_qkv": w_qkv, "b_qkv": b_qkv, "w_proj": w_proj})
    return in_maps


def kernel(hidden_states, c_attn_w, c_attn_b, c_proj_w, c_proj_b):
    nc = get_program()
    in_maps = make_in_maps(hidden_states, c_attn_w, c_attn_b, c_proj_w)
    res = run_bass_kernel_spmd(nc, in_maps, list(range(N_CORES)))
    # unshard: row-parallel projection partials sum + bias
    acc = res.results[0]["out"]
    for c in range(1, N_CORES):
        acc = acc + res.results[c]["out"]
    acc = acc + np.asarray(c_proj_b, dtype=np.float32)[None, :]
    return acc.reshape(B, S, D).astype(np.float32)


if __name__ == "__main__":
    rng = np.random.default_rng(0)
    hs = rng.standard_normal((B, S, D), dtype=np.float32)
    wa = rng.standard_normal((D, 3 * D), dtype=np.float32) * 0.02
    ba = rng.standard_normal((3 * D,), dtype=np.float32) * 0.02
    wp = rng.standard_normal((D, D), dtype=np.float32) * 0.02
    bp = rng.standard_normal((D,), dtype=np.float32) * 0.02
    out = kernel(hs, wa, ba, wp, bp)
    print("out", out.shape, out.dtype, float(np.abs(out).max()))



# revision 6
# speedup vs baseline: 1.4521x; 1.4521x over previous
"""GPT2-style fused attention (DecisionTransformer) on 8 Trainium2 NeuronCores.

Sharding: 2-D (batch x head-group).  Core c handles batch c//4 and heads
4*(c%4)..4*(c%4)+3 (4 heads, 256 of the 1024 features).  Each core:
  - loads X^T for its batch (host pre-transposes + casts to bf16, so no
    PE transposes on device and only 4 MB of X traffic per core),
  - computes Q^T/K^T (features on partitions, 2-head pairs stacked) and
    V in natural [token, feat] layout with a ones column appended, so the
    A@V matmul produces softmax denominators for free,
  - causal attention for its 4 heads: scores^T = K @ Q^T per 128-key
    block, exp on the Act engine (no max subtraction -- logits are small
    and bounded), block-causal masking on the 128x128 diagonal,
  - row-parallel output projection with its 256 rows of c_proj_w,
  - writes a full-shape partial output [2048, 1024] fp32.
Host gathers: sum the 4 partials per batch, add c_proj_b plus the folded
V-bias term (b_v @ c_proj_w).  Q/K biases are added exactly during the
QKV evictions; the V bias commutes through the softmax average so it
folds into the projection bias on the host.

All matmuls run in bf16 (1 cycle/row on the PE) with fp32 PSUM
accumulation; measured output error vs the fp32 reference is ~2e-3
relative to absmax, well within the 2e-2 gate.
"""

import sys

for _p in ("/opt/trn_rl_repo",):
    if _p not in sys.path:
        sys.path.insert(0, _p)

import numpy as np

import concourse.bass as bass
import concourse.mybir as mybir
import concourse.tile as tile
from concourse import bacc
from concourse.bass_utils import run_bass_kernel_spmd

P = 128
B, S, D, H, HD = 2, 2048, 1024, 16, 64
N_CORES = 8
GROUPS = 4              # head groups (4 heads each)
HPC = H // GROUPS       # 4 heads per core
FPC = HPC * HD          # 256 features per core (per q/k/v)
KO = D // P             # 8 contraction chunks
TCH = 512               # token chunk for qkv phase
NCH = S // TCH          # 4 chunks (and 4 query chunks)
NKB = S // P            # 16 key blocks per sequence
SCALE = 1.0 / float(HD) ** 0.5

f32 = mybir.dt.float32
f32r = mybir.dt.float32r
bf16 = mybir.dt.bfloat16


def _emit_qkv_chunk(nc, pools, consts, c):
    """QKV projection for token chunk c (512 tokens)."""
    (xt_pool, pt_pool, atn_pool, out_pool, small_pool,
     ps_s, ps_b, ps_av) = pools
    (wqkv_sb, wp_sb, bqk_sb, maskones, ones1r, qpad, ktb, vaug,
     xt_d, out_d) = consts

    cs = slice(c * TCH, (c + 1) * TCH)
    xt = xt_pool.tile([P, KO, TCH], bf16, tag="xt", name="xt")
    nc.sync.dma_start(
        xt[:], xt_d.rearrange("(ko p) t -> p ko t", p=P)[:, :, cs]
    )

    # Q^T / K^T: features on partitions (2-head pairs stacked 64+64)
    for fc in range(4):         # q01, q23, k01, k23
        ps = ps_b.tile([P, TCH], f32, tag="b", name="psb")
        for ko in range(KO):
            nc.tensor.matmul(
                ps[:],
                wqkv_sb[:, ko, fc * P : (fc + 1) * P],
                xt[:, ko, :],
                start=(ko == 0),
                stop=(ko == KO - 1),
            )
        pair = fc % 2
        if fc < 2:  # Q -> per-head zero-padded tiles, + bias
            h0, h1 = 2 * pair, 2 * pair + 1
            nc.vector.tensor_scalar(
                qpad[h0][:HD, cs], ps[:HD],
                bqk_sb[:HD, fc : fc + 1], None, mybir.AluOpType.add,
            )
            nc.vector.tensor_scalar(
                qpad[h1][HD:, cs], ps[HD:],
                bqk_sb[HD:, fc : fc + 1], None, mybir.AluOpType.add,
            )
        else:       # K -> stacked pair tile, + bias
            nc.vector.tensor_scalar(
                ktb[pair][:, cs], ps[:],
                bqk_sb[:, fc : fc + 1], None, mybir.AluOpType.add,
            )

    # V in natural [token, feat] layout (no bias -- folded on host)
    for tb in range(TCH // P):
        kb = c * (TCH // P) + tb
        ps = ps_b.tile([P, TCH], f32, tag="b", name="psb")
        for ko in range(KO):
            nc.tensor.matmul(
                ps[:, :FPC],
                xt[:, ko, tb * P : (tb + 1) * P],
                wqkv_sb[:, ko, 4 * P : 4 * P + FPC],
                start=(ko == 0),
                stop=(ko == KO - 1),
            )
        nc.vector.tensor_copy(
            vaug[:, kb, :, :HD],
            ps[:, :FPC].rearrange("p (h d) -> p h d", h=HPC),
        )


def _emit_scores(nc, pools, consts, h, qc, pts):
    """Scores + exp for head h, query chunk qc; fills pts[kb] -> pt slices."""
    (xt_pool, pt_pool, atn_pool, out_pool, small_pool,
     ps_s, ps_b, ps_av) = pools
    (wqkv_sb, wp_sb, bqk_sb, maskones, ones1r, qpad, ktb, vaug,
     xt_d, out_d) = consts

    nkb = (qc + 1) * (TCH // P)
    qs = slice(qc * TCH, (qc + 1) * TCH)
    for pr in range(nkb // 2):
        ps = ps_s.tile([P, 2 * TCH], f32, tag="s", name="pss")
        pt = pt_pool.tile([P, 2 * TCH], bf16, tag="pt", name="pt")
        los = []
        for u in range(2):
            kb = 2 * pr + u
            j = kb - qc * (TCH // P)
            lo = j * P if j > 0 else 0
            los.append((kb, j, lo))
            nc.tensor.matmul(
                ps[:, u * TCH + lo : (u + 1) * TCH],
                ktb[h // 2][:, kb * P : (kb + 1) * P],
                qpad[h][:, qc * TCH + lo : (qc + 1) * TCH],
                start=True,
                stop=True,
            )
        if los[1][1] < 0:  # both halves fully causal-valid: one wide exp
            nc.scalar.activation(
                pt[:], ps[:], mybir.ActivationFunctionType.Exp, scale=SCALE,
            )
        else:
            for u, (kb, j, lo) in enumerate(los):
                nc.scalar.activation(
                    pt[:, u * TCH + lo : (u + 1) * TCH],
                    ps[:, u * TCH + lo : (u + 1) * TCH],
                    mybir.ActivationFunctionType.Exp,
                    scale=SCALE,
                )
        for u, (kb, j, lo) in enumerate(los):
            if j >= 0:  # diagonal block: mask upper triangle
                nc.vector.tensor_tensor(
                    pt[:, u * TCH + j * P : u * TCH + (j + 1) * P],
                    pt[:, u * TCH + j * P : u * TCH + (j + 1) * P],
                    maskones[:, :P],
                    mybir.AluOpType.mult,
                )
            pts[kb] = pt[:, u * TCH : (u + 1) * TCH]


def _emit_av_norm(nc, pools, consts, h, qc, pts, atn_pair):
    """A@V accumulation + normalization for head h, query chunk qc."""
    (xt_pool, pt_pool, atn_pool, out_pool, small_pool,
     ps_s, ps_b, ps_av) = pools
    (wqkv_sb, wp_sb, bqk_sb, maskones, ones1r, qpad, ktb, vaug,
     xt_d, out_d) = consts

    nkb = (qc + 1) * (TCH // P)
    po = ps_av.tile([P, TCH], f32, tag="av", name="psav")
    for kb in range(nkb):
        j = kb - qc * (TCH // P)
        lo = j * P if j > 0 else 0
        nc.tensor.matmul(
            po[: HD + 1, lo:],
            vaug[:, kb, h, :],
            pts[kb][:, lo:],
            start=(kb == 0),
            stop=(kb == nkb - 1),
        )
    # normalize: atn = po[0:64] * (1 / po[64])  (row 64 = denominator).
    # Broadcast the denominator across 64 partitions via a rank-1 PE
    # matmul, then reciprocal on 64 lanes (a [1,512] single-lane op is
    # much slower), then scale during the PSUM eviction.
    den = small_pool.tile([1, TCH], f32r, tag="den", name="den")
    nc.vector.tensor_copy(den[:], po[HD : HD + 1, :])
    rbc = ps_b.tile([P, TCH], f32, tag="b", name="psb")
    nc.tensor.matmul(rbc[:HD, :], ones1r[:], den[:], start=True, stop=True)
    rbs = small_pool.tile([HD, TCH], f32, tag="rbs", name="rbs")
    nc.vector.reciprocal_approx_fast(out=rbs[:], in_=rbc[:HD, :])
    s = h % 2
    nc.vector.tensor_tensor(
        atn_pair[h // 2][s * HD : (s + 1) * HD, :],
        po[:HD, :],
        rbs[:],
        mybir.AluOpType.mult,
    )


def _emit_proj(nc, pools, consts, qc, atn_pair):
    """Output projection + DMA out for query chunk qc."""
    (xt_pool, pt_pool, atn_pool, out_pool, small_pool,
     ps_s, ps_b, ps_av) = pools
    (wqkv_sb, wp_sb, bqk_sb, maskones, ones1r, qpad, ktb, vaug,
     xt_d, out_d) = consts

    for tb in range(TCH // P):
        for ob in range(2):
            pp = ps_b.tile([P, TCH], f32, tag="b", name="psb")
            for fb in range(2):
                nc.tensor.matmul(
                    pp[:],
                    atn_pair[fb][:, tb * P : (tb + 1) * P],
                    wp_sb[:, fb, ob * TCH : (ob + 1) * TCH],
                    start=(fb == 0),
                    stop=(fb == 1),
                )
            ot = out_pool.tile([P, TCH], f32, tag="ot", name="ot")
            nc.vector.tensor_copy(ot[:], pp[:])
            row = qc * TCH + tb * P
            nc.sync.dma_start(
                out_d[row : row + P, ob * TCH : (ob + 1) * TCH], ot[:],
            )


def _build_program():
    nc = bacc.Bacc(None, target_bir_lowering=False)

    xt_d = nc.dram_tensor("xt", [D, S], bf16, kind="ExternalInput")
    wqkv_d = nc.dram_tensor("w_qkv", [D, 4 * P + FPC], bf16, kind="ExternalInput")
    bqk_d = nc.dram_tensor("b_qk", [4 * P], f32, kind="ExternalInput")
    wp_d = nc.dram_tensor("w_proj", [FPC, D], bf16, kind="ExternalInput")
    cst_d = nc.dram_tensor("consts", [P, P], bf16, kind="ExternalInput")
    out_d = nc.dram_tensor("out", [S, D], f32, kind="ExternalOutput")

    with tile.TileContext(nc) as tc:
        with (
            tc.tile_pool(name="const", bufs=1) as const,
            tc.tile_pool(name="xt", bufs=2) as xt_pool,
            tc.tile_pool(name="pt", bufs=24) as pt_pool,
            tc.tile_pool(name="atn", bufs=2) as atn_pool,
            tc.tile_pool(name="outp", bufs=4) as out_pool,
            tc.tile_pool(name="small", bufs=4) as small_pool,
            tc.tile_pool(name="ps_s", bufs=2, space="PSUM") as ps_s,
            tc.tile_pool(name="ps_b", bufs=2, space="PSUM") as ps_b,
            tc.tile_pool(name="ps_av", bufs=2, space="PSUM") as ps_av,
        ):
            # ---- constants (small DMAs first) ----
            # maskones[k, j] = 1.0 if k <= j else 0.0 (host-built).  Row 0 is
            # all ones, reused as the broadcast lhsT for normalization.
            maskones = const.tile([P, P], bf16)
            nc.sync.dma_start(maskones[:], cst_d[:])
            bqk_sb = const.tile([P, 4], f32)
            nc.sync.dma_start(bqk_sb[:], bqk_d.rearrange("(c p) -> p c", p=P))

            # warm up the PE pstate while the big DMAs land
            for _ in range(56):
                ps_warm = ps_s.tile([P, 2 * TCH], f32, tag="s", name="pss")
                nc.tensor.matmul(
                    ps_warm[:, :P], maskones[:], maskones[:],
                    start=True, stop=True,
                )

            wqkv_sb = const.tile([P, KO, 4 * P + FPC], bf16)
            nc.sync.dma_start(
                wqkv_sb[:], wqkv_d.rearrange("(ko p) f -> p ko f", p=P)
            )
            wp_sb = const.tile([P, 2, D], bf16)
            nc.sync.dma_start(wp_sb[:], wp_d.rearrange("(fb p) o -> p fb o", p=P))

            # persistent attention operand tiles
            qpad = [const.tile([P, S], bf16, name=f"qp{h}") for h in range(HPC)]
            ktb = [const.tile([P, S], bf16, name=f"kt{p}") for p in range(2)]
            vaug = const.tile([P, NKB, HPC, HD + 1], bf16, name="vaug")

            # zero the dead rows of qpad (other head's slot in the pair)
            for h in range(HPC):
                dead = qpad[h][HD:, :] if h % 2 == 0 else qpad[h][:HD, :]
                nc.vector.memset(dead.bitcast(f32), 0.0)
            # ones row for the 1/denominator broadcast matmul (f32r so
            # the 512-row matmul runs at 1 cycle/row)
            ones1r = const.tile([1, HD], f32r, name="ones1r")
            nc.vector.memset(ones1r[:].bitcast(f32), 1.0)
            # ones column of V_aug via f32 scratch -> bf16 strided copy
            onescr = small_pool.tile(
                [P, NKB, HPC, 1], f32, tag="ones", name="ones", bufs=1
            )
            nc.vector.memset(onescr[:], 1.0)
            nc.vector.tensor_copy(vaug[:, :, :, HD : HD + 1], onescr[:])

            pools = (xt_pool, pt_pool, atn_pool, out_pool, small_pool,
                     ps_s, ps_b, ps_av)
            consts = (wqkv_sb, wp_sb, bqk_sb, maskones, ones1r, qpad,
                      ktb, vaug, xt_d, out_d)

            # ---- pipeline: QKV(c) -> attn(qc=c) -> proj(qc=c) ----
            # emission order = per-engine execution order; QKV(c+1) is
            # emitted before proj(c) so the PE never waits on the exp chain
            prev_proj = None
            for c in range(NCH):
                _emit_qkv_chunk(nc, pools, consts, c)
                if prev_proj is not None:
                    _emit_proj(nc, pools, consts, *prev_proj)
                atn_pair = [
                    atn_pool.tile([P, TCH], bf16, tag=f"atn{p}", name=f"atn{p}")
                    for p in range(2)
                ]
                pts = [{} for _ in range(HPC)]
                # 2-deep score lookahead so the Act engine's exp backlog
                # never stalls the PE's A@V stream
                _emit_scores(nc, pools, consts, 0, c, pts[0])
                _emit_scores(nc, pools, consts, 1, c, pts[1])
                _emit_av_norm(nc, pools, consts, 0, c, pts[0], atn_pair)
                _emit_scores(nc, pools, consts, 2, c, pts[2])
                _emit_av_norm(nc, pools, consts, 1, c, pts[1], atn_pair)
                _emit_scores(nc, pools, consts, 3, c, pts[3])
                _emit_av_norm(nc, pools, consts, 2, c, pts[2], atn_pair)
                _emit_av_norm(nc, pools, consts, 3, c, pts[3], atn_pair)
                prev_proj = (c, atn_pair)
            _emit_proj(nc, pools, consts, *prev_proj)

    nc.compile()
    return nc


_CACHE = {}


def get_program():
    if "nc" not in _CACHE:
        _CACHE["nc"] = _build_program()
    return _CACHE["nc"]


def make_in_maps(hidden_states, c_attn_w, c_attn_b, c_proj_w):
    import ml_dtypes

    bf = ml_dtypes.bfloat16
    x = np.asarray(hidden_states, dtype=np.float32)
    wa = np.asarray(c_attn_w, dtype=np.float32)
    ba = np.asarray(c_attn_b, dtype=np.float32)
    wp = np.asarray(c_proj_w, dtype=np.float32)

    xts = [np.ascontiguousarray(x[b].T).astype(bf) for b in range(B)]
    m = np.tril(np.ones((P, P), dtype=np.float32)).T  # m[k, j] = k <= j
    consts = np.ascontiguousarray(m).astype(bf)

    in_maps = []
    for c in range(N_CORES):
        b, g = divmod(c, GROUPS)
        lo, hi = g * FPC, (g + 1) * FPC
        w_qkv = np.ascontiguousarray(
            np.concatenate(
                [wa[:, lo:hi], wa[:, D + lo : D + hi], wa[:, 2 * D + lo : 2 * D + hi]],
                axis=1,
            )
        ).astype(bf)
        b_qk = np.ascontiguousarray(
            np.concatenate([ba[lo:hi], ba[D + lo : D + hi]])
        ).astype(np.float32)
        w_proj = np.ascontiguousarray(wp[lo:hi, :]).astype(bf)
        in_maps.append({
            "xt": xts[b],
            "w_qkv": w_qkv,
            "b_qk": b_qk,
            "w_proj": w_proj,
            "consts": consts,
        })
    return in_maps


def kernel(hidden_states, c_attn_w, c_attn_b, c_proj_w, c_proj_b):
    nc = get_program()
    in_maps = make_in_maps(hidden_states, c_attn_w, c_attn_b, c_proj_w)
    res = run_bass_kernel_spmd(nc, in_maps, list(range(N_CORES)))
    # unshard: sum the 4 head-group partials per batch; V bias commutes
    # through the attention average, so it folds into the proj bias here.
    ba = np.asarray(c_attn_b, dtype=np.float32)
    bias = np.asarray(c_proj_b, dtype=np.float32) + ba[2 * D :] @ np.asarray(
        c_proj_w, dtype=np.float32
    )
    out = np.empty((B, S, D), dtype=np.float32)
    for b in range(B):
        acc = res.results[4 * b]["out"]
        for g in range(1, GROUPS):
            acc = acc + res.results[4 * b + g]["out"]
        out[b] = acc + bias[None, :]
    return out


if __name__ == "__main__":
    rng = np.random.default_rng(0)
    hs = rng.standard_normal((B, S, D), dtype=np.float32)
    wa = rng.standard_normal((D, 3 * D), dtype=np.float32) * 0.02
    ba = rng.standard_normal((3 * D,), dtype=np.float32) * 0.02
    wp = rng.standard_normal((D, D), dtype=np.float32) * 0.02
    bp = rng.standard_normal((D,), dtype=np.float32) * 0.02
    out = kernel(hs, wa, ba, wp, bp)
    print("out", out.shape, out.dtype, float(np.abs(out).max()))


# revision 8
# speedup vs baseline: 1.5708x; 1.0817x over previous
"""GPT2-style fused attention (DecisionTransformer) on 8 Trainium2 NeuronCores.

Sharding: 2-D (batch x head-group).  Core c handles batch c//4 and heads
4*(c%4)..4*(c%4)+3 (4 heads, 256 of the 1024 features).  Each core:
  - loads X^T for its batch (host pre-transposes + casts to bf16, so no
    PE transposes on device and only 4 MB of X traffic per core),
  - computes Q^T/K^T (features on partitions, 2-head pairs stacked) and
    V in natural [token, feat] layout with a ones column appended, so the
    A@V matmul produces softmax denominators for free,
  - causal attention for its 4 heads: scores^T = K @ Q^T per 128-key
    block, exp on the Act engine (no max subtraction -- logits are small
    and bounded), block-causal masking on the 128x128 diagonal,
  - row-parallel output projection with its 256 rows of c_proj_w,
  - writes a full-shape partial output [2048, 1024] fp32.
Host gathers: sum the 4 partials per batch, add c_proj_b plus the folded
V-bias term (b_v @ c_proj_w).  Q/K biases are added exactly during the
QKV evictions; the V bias commutes through the softmax average so it
folds into the projection bias on the host.

All matmuls run in bf16 (1 cycle/row on the PE) with fp32 PSUM
accumulation; measured output error vs the fp32 reference is ~2e-3
relative to absmax, well within the 2e-2 gate.
"""

import sys

for _p in ("/opt/trn_rl_repo",):
    if _p not in sys.path:
        sys.path.insert(0, _p)

import numpy as np

import concourse.bass as bass
import concourse.mybir as mybir
import concourse.tile as tile
from concourse import bacc
from concourse.bass_utils import run_bass_kernel_spmd

P = 128
B, S, D, H, HD = 2, 2048, 1024, 16, 64
N_CORES = 8
GROUPS = 4              # head groups (4 heads each)
HPC = H // GROUPS       # 4 heads per core
FPC = HPC * HD          # 256 features per core (per q/k/v)
KO = D // P             # 8 contraction chunks
TCH = 512               # token chunk for qkv phase
NCH = S // TCH          # 4 chunks (and 4 query chunks)
NKB = S // P            # 16 key blocks per sequence
SCALE = 1.0 / float(HD) ** 0.5

f32 = mybir.dt.float32
f32r = mybir.dt.float32r
bf16 = mybir.dt.bfloat16


def _load_xt_chunk(nc, xt_pool, xt_d, c):
    cs = slice(c * TCH, (c + 1) * TCH)
    xt = xt_pool.tile([P, KO, TCH], bf16, tag="xt", name="xt")
    nc.sync.dma_start(
        xt[:], xt_d.rearrange("(ko p) t -> p ko t", p=P)[:, :, cs]
    )
    return xt


def _emit_qkv_chunk(nc, pools, consts, c, xt=None):
    """QKV projection for token chunk c (512 tokens)."""
    (xt_pool, pt_pool, atn_pool, out_pool, small_pool,
     ps_s, ps_b, ps_av) = pools
    (wqkv_sb, wp_sb, bqk_sb, maskones, ones1r, qpad, ktb, vaug,
     xt_d, out_d) = consts

    cs = slice(c * TCH, (c + 1) * TCH)
    if xt is None:
        xt = _load_xt_chunk(nc, xt_pool, xt_d, c)

    # Q^T / K^T: features on partitions (2-head pairs stacked 64+64)
    for fc in range(4):         # q01, q23, k01, k23
        ps = ps_b.tile([P, TCH], f32, tag="b", name="psb")
        for ko in range(KO):
            nc.tensor.matmul(
                ps[:],
                wqkv_sb[:, ko, fc * P : (fc + 1) * P],
                xt[:, ko, :],
                start=(ko == 0),
                stop=(ko == KO - 1),
            )
        pair = fc % 2
        if fc < 2:  # Q -> per-head zero-padded tiles, + bias
            h0, h1 = 2 * pair, 2 * pair + 1
            nc.vector.tensor_scalar(
                qpad[h0][:HD, cs], ps[:HD],
                bqk_sb[:HD, fc : fc + 1], None, mybir.AluOpType.add,
            )
            nc.vector.tensor_scalar(
                qpad[h1][HD:, cs], ps[HD:],
                bqk_sb[HD:, fc : fc + 1], None, mybir.AluOpType.add,
            )
        else:       # K -> stacked pair tile, + bias
            nc.vector.tensor_scalar(
                ktb[pair][:, cs], ps[:],
                bqk_sb[:, fc : fc + 1], None, mybir.AluOpType.add,
            )

    # V in natural [token, feat] layout (no bias -- folded on host)
    for tb in range(TCH // P):
        kb = c * (TCH // P) + tb
        ps = ps_b.tile([P, TCH], f32, tag="b", name="psb")
        for ko in range(KO):
            nc.tensor.matmul(
                ps[:, :FPC],
                xt[:, ko, tb * P : (tb + 1) * P],
                wqkv_sb[:, ko, 4 * P : 4 * P + FPC],
                start=(ko == 0),
                stop=(ko == KO - 1),
            )
        nc.vector.tensor_copy(
            vaug[:, kb, :, :HD],
            ps[:, :FPC].rearrange("p (h d) -> p h d", h=HPC),
        )


def _emit_scores(nc, pools, consts, h, qc, pts):
    """Scores + exp for head h, query chunk qc; fills pts[kb] -> pt slices."""
    (xt_pool, pt_pool, atn_pool, out_pool, small_pool,
     ps_s, ps_b, ps_av) = pools
    (wqkv_sb, wp_sb, bqk_sb, maskones, ones1r, qpad, ktb, vaug,
     xt_d, out_d) = consts

    nkb = (qc + 1) * (TCH // P)
    qs = slice(qc * TCH, (qc + 1) * TCH)
    for pr in range(nkb // 2):
        ps = ps_s.tile([P, 2 * TCH], f32, tag="s", name="pss")
        pt = pt_pool.tile([P, 2 * TCH], bf16, tag="pt", name="pt")
        los = []
        for u in range(2):
            kb = 2 * pr + u
            j = kb - qc * (TCH // P)
            lo = j * P if j > 0 else 0
            los.append((kb, j, lo))
            nc.tensor.matmul(
                ps[:, u * TCH + lo : (u + 1) * TCH],
                ktb[h // 2][:, kb * P : (kb + 1) * P],
                qpad[h][:, qc * TCH + lo : (qc + 1) * TCH],
                start=True,
                stop=True,
            )
        if los[1][1] < 0:  # both halves fully causal-valid: one wide exp
            nc.scalar.activation(
                pt[:], ps[:], mybir.ActivationFunctionType.Exp, scale=SCALE,
            )
        else:
            for u, (kb, j, lo) in enumerate(los):
                nc.scalar.activation(
                    pt[:, u * TCH + lo : (u + 1) * TCH],
                    ps[:, u * TCH + lo : (u + 1) * TCH],
                    mybir.ActivationFunctionType.Exp,
                    scale=SCALE,
                )
        for u, (kb, j, lo) in enumerate(los):
            if j >= 0:  # diagonal block: mask upper triangle
                nc.vector.tensor_tensor(
                    pt[:, u * TCH + j * P : u * TCH + (j + 1) * P],
                    pt[:, u * TCH + j * P : u * TCH + (j + 1) * P],
                    maskones[:, :P],
                    mybir.AluOpType.mult,
                )
            pts[kb] = pt[:, u * TCH : (u + 1) * TCH]


def _emit_av_norm(nc, pools, consts, h, qc, pts, atn_pair):
    """A@V accumulation + normalization for head h, query chunk qc."""
    (xt_pool, pt_pool, atn_pool, out_pool, small_pool,
     ps_s, ps_b, ps_av) = pools
    (wqkv_sb, wp_sb, bqk_sb, maskones, ones1r, qpad, ktb, vaug,
     xt_d, out_d) = consts

    nkb = (qc + 1) * (TCH // P)
    po = ps_av.tile([P, TCH], f32, tag="av", name="psav")
    for kb in range(nkb):
        j = kb - qc * (TCH // P)
        lo = j * P if j > 0 else 0
        nc.tensor.matmul(
            po[: HD + 1, lo:],
            vaug[:, kb, h, :],
            pts[kb][:, lo:],
            start=(kb == 0),
            stop=(kb == nkb - 1),
        )
    # normalize: atn = po[0:64] * (1 / po[64])  (row 64 = denominator).
    # Broadcast the denominator across 64 partitions via a rank-1 PE
    # matmul, then reciprocal on 64 lanes (a [1,512] single-lane op is
    # much slower), then scale during the PSUM eviction.
    den = small_pool.tile([1, TCH], f32r, tag="den", name="den")
    nc.vector.tensor_copy(den[:], po[HD : HD + 1, :])
    rbc = ps_b.tile([P, TCH], f32, tag="b", name="psb")
    nc.tensor.matmul(rbc[:HD, :], ones1r[:], den[:], start=True, stop=True)
    rbs = small_pool.tile([HD, TCH], f32, tag="rbs", name="rbs")
    nc.vector.reciprocal_approx_fast(out=rbs[:], in_=rbc[:HD, :])
    s = h % 2
    nc.vector.tensor_tensor(
        atn_pair[h // 2][s * HD : (s + 1) * HD, :],
        po[:HD, :],
        rbs[:],
        mybir.AluOpType.mult,
    )


def _emit_proj(nc, pools, consts, qc, atn_pair):
    """Output projection + DMA out for query chunk qc."""
    (xt_pool, pt_pool, atn_pool, out_pool, small_pool,
     ps_s, ps_b, ps_av) = pools
    (wqkv_sb, wp_sb, bqk_sb, maskones, ones1r, qpad, ktb, vaug,
     xt_d, out_d) = consts

    for tb in range(TCH // P):
        for ob in range(2):
            pp = ps_b.tile([P, TCH], f32, tag="b", name="psb")
            for fb in range(2):
                nc.tensor.matmul(
                    pp[:],
                    atn_pair[fb][:, tb * P : (tb + 1) * P],
                    wp_sb[:, fb, ob * TCH : (ob + 1) * TCH],
                    start=(fb == 0),
                    stop=(fb == 1),
                )
            ot = out_pool.tile([P, TCH], f32, tag="ot", name="ot")
            nc.vector.tensor_copy(ot[:], pp[:])
            row = qc * TCH + tb * P
            nc.sync.dma_start(
                out_d[row : row + P, ob * TCH : (ob + 1) * TCH], ot[:],
            )


def _build_program():
    nc = bacc.Bacc(None, target_bir_lowering=False)

    xt_d = nc.dram_tensor("xt", [D, S], bf16, kind="ExternalInput")
    wqkv_d = nc.dram_tensor("w_qkv", [D, 4 * P + FPC], bf16, kind="ExternalInput")
    bqk_d = nc.dram_tensor("b_qk", [4 * P], f32, kind="ExternalInput")
    wp_d = nc.dram_tensor("w_proj", [FPC, D], bf16, kind="ExternalInput")
    cst_d = nc.dram_tensor("consts", [P, P], bf16, kind="ExternalInput")
    out_d = nc.dram_tensor("out", [S, D], f32, kind="ExternalOutput")

    with tile.TileContext(nc) as tc:
        with (
            tc.tile_pool(name="const", bufs=1) as const,
            tc.tile_pool(name="xt", bufs=2) as xt_pool,
            tc.tile_pool(name="pt", bufs=24) as pt_pool,
            tc.tile_pool(name="atn", bufs=2) as atn_pool,
            tc.tile_pool(name="outp", bufs=4) as out_pool,
            tc.tile_pool(name="small", bufs=4) as small_pool,
            tc.tile_pool(name="ps_s", bufs=2, space="PSUM") as ps_s,
            tc.tile_pool(name="ps_b", bufs=2, space="PSUM") as ps_b,
            tc.tile_pool(name="ps_av", bufs=2, space="PSUM") as ps_av,
        ):
            # ---- constants (small DMAs first) ----
            # maskones[k, j] = 1.0 if k <= j else 0.0 (host-built).  Row 0 is
            # all ones, reused as the broadcast lhsT for normalization.
            maskones = const.tile([P, P], bf16)
            nc.sync.dma_start(maskones[:], cst_d[:])
            bqk_sb = const.tile([P, 4], f32)
            nc.sync.dma_start(bqk_sb[:], bqk_d.rearrange("(c p) -> p c", p=P))

            # prefetch the first token chunk before the big weight DMAs
            xt0 = _load_xt_chunk(nc, xt_pool, xt_d, 0)

            # warm up the PE pstate while the big DMAs land
            for _ in range(56):
                ps_warm = ps_s.tile([P, 2 * TCH], f32, tag="s", name="pss")
                nc.tensor.matmul(
                    ps_warm[:, :P], maskones[:], maskones[:],
                    start=True, stop=True,
                )

            wqkv_sb = const.tile([P, KO, 4 * P + FPC], bf16)
            nc.sync.dma_start(
                wqkv_sb[:], wqkv_d.rearrange("(ko p) f -> p ko f", p=P)
            )
            wp_sb = const.tile([P, 2, D], bf16)
            nc.sync.dma_start(wp_sb[:], wp_d.rearrange("(fb p) o -> p fb o", p=P))

            # persistent attention operand tiles
            qpad = [const.tile([P, S], bf16, name=f"qp{h}") for h in range(HPC)]
            ktb = [const.tile([P, S], bf16, name=f"kt{p}") for p in range(2)]
            vaug = const.tile([P, NKB, HPC, HD + 1], bf16, name="vaug")

            # zero the dead rows of qpad (other head's slot in the pair)
            for h in range(HPC):
                dead = qpad[h][HD:, :] if h % 2 == 0 else qpad[h][:HD, :]
                nc.vector.memset(dead.bitcast(f32), 0.0)
            # ones row for the 1/denominator broadcast matmul (f32r so
            # the 512-row matmul runs at 1 cycle/row)
            ones1r = const.tile([1, HD], f32r, name="ones1r")
            nc.vector.memset(ones1r[:].bitcast(f32), 1.0)
            # ones column of V_aug via f32 scratch -> bf16 strided copy
            onescr = small_pool.tile(
                [P, NKB, HPC, 1], f32, tag="ones", name="ones", bufs=1
            )
            nc.vector.memset(onescr[:], 1.0)
            nc.vector.tensor_copy(vaug[:, :, :, HD : HD + 1], onescr[:])

            pools = (xt_pool, pt_pool, atn_pool, out_pool, small_pool,
                     ps_s, ps_b, ps_av)
            consts = (wqkv_sb, wp_sb, bqk_sb, maskones, ones1r, qpad,
                      ktb, vaug, xt_d, out_d)

            # ---- pipeline: QKV(c+1) is emitted BEFORE attn(c) so the PE's
            # in-order queue always has a dependency-free QKV chunk to chew
            # on while the Act engine works through attn(c)'s exp chain;
            # proj(c) (which waits on the whole attn chain) goes after the
            # next QKV chunk for the same reason.
            _emit_qkv_chunk(nc, pools, consts, 0, xt=xt0)
            prev_proj = None
            for c in range(NCH):
                if c + 1 < NCH:
                    _emit_qkv_chunk(nc, pools, consts, c + 1)
                if prev_proj is not None:
                    _emit_proj(nc, pools, consts, *prev_proj)
                atn_pair = [
                    atn_pool.tile([P, TCH], bf16, tag=f"atn{p}", name=f"atn{p}")
                    for p in range(2)
                ]
                pts = [{} for _ in range(HPC)]
                # 2-deep score lookahead so the Act engine's exp backlog
                # never stalls the PE's A@V stream
                _emit_scores(nc, pools, consts, 0, c, pts[0])
                _emit_scores(nc, pools, consts, 1, c, pts[1])
                _emit_av_norm(nc, pools, consts, 0, c, pts[0], atn_pair)
                _emit_scores(nc, pools, consts, 2, c, pts[2])
                _emit_av_norm(nc, pools, consts, 1, c, pts[1], atn_pair)
                _emit_scores(nc, pools, consts, 3, c, pts[3])
                _emit_av_norm(nc, pools, consts, 2, c, pts[2], atn_pair)
                _emit_av_norm(nc, pools, consts, 3, c, pts[3], atn_pair)
                prev_proj = (c, atn_pair)
            _emit_proj(nc, pools, consts, *prev_proj)

    nc.compile()
    return nc


_CACHE = {}


def get_program():
    if "nc" not in _CACHE:
        _CACHE["nc"] = _build_program()
    return _CACHE["nc"]


def make_in_maps(hidden_states, c_attn_w, c_attn_b, c_proj_w):
    import ml_dtypes

    bf = ml_dtypes.bfloat16
    x = np.asarray(hidden_states, dtype=np.float32)
    wa = np.asarray(c_attn_w, dtype=np.float32)
    ba = np.asarray(c_attn_b, dtype=np.float32)
    wp = np.asarray(c_proj_w, dtype=np.float32)

    xts = [np.ascontiguousarray(x[b].T).astype(bf) for b in range(B)]
    m = np.tril(np.ones((P, P), dtype=np.float32)).T  # m[k, j] = k <= j
    consts = np.ascontiguousarray(m).astype(bf)

    in_maps = []
    for c in range(N_CORES):
        b, g = divmod(c, GROUPS)
        lo, hi = g * FPC, (g + 1) * FPC
        w_qkv = np.ascontiguousarray(
            np.concatenate(
                [wa[:, lo:hi], wa[:, D + lo : D + hi], wa[:, 2 * D + lo : 2 * D + hi]],
                axis=1,
            )
        ).astype(bf)
        b_qk = np.ascontiguousarray(
            np.concatenate([ba[lo:hi], ba[D + lo : D + hi]])
        ).astype(np.float32)
        w_proj = np.ascontiguousarray(wp[lo:hi, :]).astype(bf)
        in_maps.append({
            "xt": xts[b],
            "w_qkv": w_qkv,
            "b_qk": b_qk,
            "w_proj": w_proj,
            "consts": consts,
        })
    return in_maps


def kernel(hidden_states, c_attn_w, c_attn_b, c_proj_w, c_proj_b):
    nc = get_program()
    in_maps = make_in_maps(hidden_states, c_attn_w, c_attn_b, c_proj_w)
    res = run_bass_kernel_spmd(nc, in_maps, list(range(N_CORES)))
    # unshard: sum the 4 head-group partials per batch; V bias commutes
    # through the attention average, so it folds into the proj bias here.
    ba = np.asarray(c_attn_b, dtype=np.float32)
    bias = np.asarray(c_proj_b, dtype=np.float32) + ba[2 * D :] @ np.asarray(
        c_proj_w, dtype=np.float32
    )
    out = np.empty((B, S, D), dtype=np.float32)
    for b in range(B):
        acc = res.results[4 * b]["out"]
        for g in range(1, GROUPS):
            acc = acc + res.results[4 * b + g]["out"]
        out[b] = acc + bias[None, :]
    return out


if __name__ == "__main__":
    rng = np.random.default_rng(0)
    hs = rng.standard_normal((B, S, D), dtype=np.float32)
    wa = rng.standard_normal((D, 3 * D), dtype=np.float32) * 0.02
    ba = rng.standard_normal((3 * D,), dtype=np.float32) * 0.02
    wp = rng.standard_normal((D, D), dtype=np.float32) * 0.02
    bp = rng.standard_normal((D,), dtype=np.float32) * 0.02
    out = kernel(hs, wa, ba, wp, bp)
    print("out", out.shape, out.dtype, float(np.abs(out).max()))


# revision 10
# speedup vs baseline: 1.6542x; 1.0531x over previous
"""GPT2-style fused attention (DecisionTransformer) on 8 Trainium2 NeuronCores.

Sharding: 2-D (batch x head-group).  Core c handles batch c//4 and heads
4*(c%4)..4*(c%4)+3 (4 heads, 256 of the 1024 features).  Each core:
  - loads X^T for its batch (host pre-transposes + casts to bf16, so no
    PE transposes on device and only 4 MB of X traffic per core),
  - computes Q^T/K^T (features on partitions, 2-head pairs stacked) and
    V in natural [token, feat] layout with a ones column appended, so the
    A@V matmul produces softmax denominators for free,
  - causal attention for its 4 heads: scores^T = K @ Q^T per 128-key
    block, exp on the Act engine (no max subtraction -- logits are small
    and bounded), block-causal masking on the 128x128 diagonal,
  - row-parallel output projection with its 256 rows of c_proj_w,
  - writes a full-shape partial output [2048, 1024] fp32.
Host gathers: sum the 4 partials per batch, add c_proj_b plus the folded
V-bias term (b_v @ c_proj_w).  Q/K biases are added exactly during the
QKV evictions; the V bias commutes through the softmax average so it
folds into the projection bias on the host.

All matmuls run in bf16 (1 cycle/row on the PE) with fp32 PSUM
accumulation; measured output error vs the fp32 reference is ~2e-3
relative to absmax, well within the 2e-2 gate.
"""

import sys

for _p in ("/opt/trn_rl_repo",):
    if _p not in sys.path:
        sys.path.insert(0, _p)

import numpy as np

import concourse.bass as bass
import concourse.mybir as mybir
import concourse.tile as tile
from concourse import bacc
from concourse.bass_utils import run_bass_kernel_spmd

P = 128
B, S, D, H, HD = 2, 2048, 1024, 16, 64
N_CORES = 8
GROUPS = 4              # head groups (4 heads each)
HPC = H // GROUPS       # 4 heads per core
FPC = HPC * HD          # 256 features per core (per q/k/v)
KO = D // P             # 8 contraction chunks
TCH = 512               # token chunk for qkv phase
NCH = S // TCH          # 4 chunks (and 4 query chunks)
NKB = S // P            # 16 key blocks per sequence
SCALE = 1.0 / float(HD) ** 0.5

f32 = mybir.dt.float32
f32r = mybir.dt.float32r
bf16 = mybir.dt.bfloat16


def _load_xt_chunk(nc, xt_pool, xt_d, c):
    cs = slice(c * TCH, (c + 1) * TCH)
    xt = xt_pool.tile([P, KO, TCH], bf16, tag="xt", name="xt")
    nc.sync.dma_start(
        xt[:], xt_d.rearrange("(ko p) t -> p ko t", p=P)[:, :, cs]
    )
    return xt


def _qkv_units(nc, pools, consts, c, xt=None):
    """QKV projection for token chunk c (512 tokens), as a list of
    emission-unit closures (4 q/k feature groups + 2 V token-block pairs)
    so the scheduler can weave them between attention sub-bursts."""
    (xt_pool, pt_pool, atn_pool, out_pool, small_pool,
     ps_s, ps_b, ps_av) = pools
    (wqkv_sb, wp_sb, bqk_sb, maskones, ones1r, qpad, ktb, vaug,
     xt_d, out_d) = consts

    cs = slice(c * TCH, (c + 1) * TCH)
    if xt is None:
        xt = _load_xt_chunk(nc, xt_pool, xt_d, c)

    def qk_unit(fc):
        def emit():
            ps = ps_b.tile([P, TCH], f32, tag="b", name="psb")
            for ko in range(KO):
                nc.tensor.matmul(
                    ps[:],
                    wqkv_sb[:, ko, fc * P : (fc + 1) * P],
                    xt[:, ko, :],
                    start=(ko == 0),
                    stop=(ko == KO - 1),
                )
            pair = fc % 2
            if fc < 2:  # Q -> per-head zero-padded tiles, + bias
                h0, h1 = 2 * pair, 2 * pair + 1
                nc.vector.tensor_scalar(
                    qpad[h0][:HD, cs], ps[:HD],
                    bqk_sb[:HD, fc : fc + 1], None, mybir.AluOpType.add,
                )
                nc.vector.tensor_scalar(
                    qpad[h1][HD:, cs], ps[HD:],
                    bqk_sb[HD:, fc : fc + 1], None, mybir.AluOpType.add,
                )
            else:       # K -> stacked pair tile, + bias
                nc.vector.tensor_scalar(
                    ktb[pair][:, cs], ps[:],
                    bqk_sb[:, fc : fc + 1], None, mybir.AluOpType.add,
                )
        return emit

    def v_unit(tb0):
        def emit():
            # V in natural [token, feat] layout (no bias -- folded on host)
            for tb in (tb0, tb0 + 1):
                kb = c * (TCH // P) + tb
                ps = ps_b.tile([P, TCH], f32, tag="b", name="psb")
                for ko in range(KO):
                    nc.tensor.matmul(
                        ps[:, :FPC],
                        xt[:, ko, tb * P : (tb + 1) * P],
                        wqkv_sb[:, ko, 4 * P : 4 * P + FPC],
                        start=(ko == 0),
                        stop=(ko == KO - 1),
                    )
                nc.vector.tensor_copy(
                    vaug[:, kb, :, :HD],
                    ps[:, :FPC].rearrange("p (h d) -> p h d", h=HPC),
                )
        return emit

    return [qk_unit(fc) for fc in range(4)] + [v_unit(0), v_unit(2)]


def _emit_scores(nc, pools, consts, h, qc, pts):
    """Scores + exp for head h, query chunk qc; fills pts[kb] -> pt slices."""
    (xt_pool, pt_pool, atn_pool, out_pool, small_pool,
     ps_s, ps_b, ps_av) = pools
    (wqkv_sb, wp_sb, bqk_sb, maskones, ones1r, qpad, ktb, vaug,
     xt_d, out_d) = consts

    nkb = (qc + 1) * (TCH // P)
    qs = slice(qc * TCH, (qc + 1) * TCH)
    for pr in range(nkb // 2):
        ps = ps_s.tile([P, 2 * TCH], f32, tag="s", name="pss")
        pt = pt_pool.tile([P, 2 * TCH], bf16, tag="pt", name="pt")
        los = []
        for u in range(2):
            kb = 2 * pr + u
            j = kb - qc * (TCH // P)
            lo = j * P if j > 0 else 0
            los.append((kb, j, lo))
            nc.tensor.matmul(
                ps[:, u * TCH + lo : (u + 1) * TCH],
                ktb[h // 2][:, kb * P : (kb + 1) * P],
                qpad[h][:, qc * TCH + lo : (qc + 1) * TCH],
                start=True,
                stop=True,
            )
        if los[1][1] < 0:  # both halves fully causal-valid: one wide exp
            nc.scalar.activation(
                pt[:], ps[:], mybir.ActivationFunctionType.Exp, scale=SCALE,
            )
        else:
            for u, (kb, j, lo) in enumerate(los):
                nc.scalar.activation(
                    pt[:, u * TCH + lo : (u + 1) * TCH],
                    ps[:, u * TCH + lo : (u + 1) * TCH],
                    mybir.ActivationFunctionType.Exp,
                    scale=SCALE,
                )
        for u, (kb, j, lo) in enumerate(los):
            if j >= 0:  # diagonal block: mask upper triangle
                nc.vector.tensor_tensor(
                    pt[:, u * TCH + j * P : u * TCH + (j + 1) * P],
                    pt[:, u * TCH + j * P : u * TCH + (j + 1) * P],
                    maskones[:, :P],
                    mybir.AluOpType.mult,
                )
            pts[kb] = pt[:, u * TCH : (u + 1) * TCH]


def _emit_av_norm(nc, pools, consts, h, qc, pts, atn_pair):
    """A@V accumulation + normalization for head h, query chunk qc."""
    (xt_pool, pt_pool, atn_pool, out_pool, small_pool,
     ps_s, ps_b, ps_av) = pools
    (wqkv_sb, wp_sb, bqk_sb, maskones, ones1r, qpad, ktb, vaug,
     xt_d, out_d) = consts

    nkb = (qc + 1) * (TCH // P)
    po = ps_av.tile([P, TCH], f32, tag="av", name="psav")
    for kb in range(nkb):
        j = kb - qc * (TCH // P)
        lo = j * P if j > 0 else 0
        nc.tensor.matmul(
            po[: HD + 1, lo:],
            vaug[:, kb, h, :],
            pts[kb][:, lo:],
            start=(kb == 0),
            stop=(kb == nkb - 1),
        )
    # normalize: atn = po[0:64] * (1 / po[64])  (row 64 = denominator).
    # Broadcast the denominator across 64 partitions via a rank-1 PE
    # matmul, then reciprocal on 64 lanes (a [1,512] single-lane op is
    # much slower), then scale during the PSUM eviction.
    den = small_pool.tile([1, TCH], f32r, tag="den", name="den")
    nc.vector.tensor_copy(den[:], po[HD : HD + 1, :])
    rbc = ps_b.tile([P, TCH], f32, tag="b", name="psb")
    nc.tensor.matmul(rbc[:HD, :], ones1r[:], den[:], start=True, stop=True)
    rbs = small_pool.tile([HD, TCH], f32, tag="rbs", name="rbs")
    nc.vector.reciprocal_approx_fast(out=rbs[:], in_=rbc[:HD, :])
    s = h % 2
    nc.vector.tensor_tensor(
        atn_pair[h // 2][s * HD : (s + 1) * HD, :],
        po[:HD, :],
        rbs[:],
        mybir.AluOpType.mult,
    )


def _proj_units(nc, pools, consts, qc, atn_pair):
    """Output projection + DMA out for query chunk qc, as 4 per-token-block
    emission units."""
    (xt_pool, pt_pool, atn_pool, out_pool, small_pool,
     ps_s, ps_b, ps_av) = pools
    (wqkv_sb, wp_sb, bqk_sb, maskones, ones1r, qpad, ktb, vaug,
     xt_d, out_d) = consts

    def unit(tb):
        def emit():
            for ob in range(2):
                pp = ps_b.tile([P, TCH], f32, tag="b", name="psb")
                for fb in range(2):
                    nc.tensor.matmul(
                        pp[:],
                        atn_pair[fb][:, tb * P : (tb + 1) * P],
                        wp_sb[:, fb, ob * TCH : (ob + 1) * TCH],
                        start=(fb == 0),
                        stop=(fb == 1),
                    )
                ot = out_pool.tile([P, TCH], f32, tag="ot", name="ot")
                nc.vector.tensor_copy(ot[:], pp[:])
                row = qc * TCH + tb * P
                nc.sync.dma_start(
                    out_d[row : row + P, ob * TCH : (ob + 1) * TCH], ot[:],
                )
        return emit

    return [unit(tb) for tb in range(TCH // P)]


def _build_program():
    nc = bacc.Bacc(None, target_bir_lowering=False)

    xt_d = nc.dram_tensor("xt", [D, S], bf16, kind="ExternalInput")
    wqkv_d = nc.dram_tensor("w_qkv", [D, 4 * P + FPC], bf16, kind="ExternalInput")
    bqk_d = nc.dram_tensor("b_qk", [4 * P], f32, kind="ExternalInput")
    wp_d = nc.dram_tensor("w_proj", [FPC, D], bf16, kind="ExternalInput")
    cst_d = nc.dram_tensor("consts", [P, P], bf16, kind="ExternalInput")
    out_d = nc.dram_tensor("out", [S, D], f32, kind="ExternalOutput")

    with tile.TileContext(nc) as tc:
        with (
            tc.tile_pool(name="const", bufs=1) as const,
            tc.tile_pool(name="xt", bufs=2) as xt_pool,
            tc.tile_pool(name="pt", bufs=24) as pt_pool,
            tc.tile_pool(name="atn", bufs=2) as atn_pool,
            tc.tile_pool(name="outp", bufs=4) as out_pool,
            tc.tile_pool(name="small", bufs=4) as small_pool,
            tc.tile_pool(name="ps_s", bufs=2, space="PSUM") as ps_s,
            tc.tile_pool(name="ps_b", bufs=2, space="PSUM") as ps_b,
            tc.tile_pool(name="ps_av", bufs=2, space="PSUM") as ps_av,
        ):
            # ---- constants (small DMAs first) ----
            # maskones[k, j] = 1.0 if k <= j else 0.0 (host-built).  Row 0 is
            # all ones, reused as the broadcast lhsT for normalization.
            maskones = const.tile([P, P], bf16)
            nc.sync.dma_start(maskones[:], cst_d[:])
            bqk_sb = const.tile([P, 4], f32)
            nc.sync.dma_start(bqk_sb[:], bqk_d.rearrange("(c p) -> p c", p=P))

            # prefetch the first token chunk before the big weight DMAs
            xt0 = _load_xt_chunk(nc, xt_pool, xt_d, 0)

            # warm up the PE pstate while the big DMAs land
            for _ in range(56):
                ps_warm = ps_s.tile([P, 2 * TCH], f32, tag="s", name="pss")
                nc.tensor.matmul(
                    ps_warm[:, :P], maskones[:], maskones[:],
                    start=True, stop=True,
                )

            wqkv_sb = const.tile([P, KO, 4 * P + FPC], bf16)
            nc.sync.dma_start(
                wqkv_sb[:], wqkv_d.rearrange("(ko p) f -> p ko f", p=P)
            )
            wp_sb = const.tile([P, 2, D], bf16)
            nc.sync.dma_start(wp_sb[:], wp_d.rearrange("(fb p) o -> p fb o", p=P))

            # persistent attention operand tiles
            qpad = [const.tile([P, S], bf16, name=f"qp{h}") for h in range(HPC)]
            ktb = [const.tile([P, S], bf16, name=f"kt{p}") for p in range(2)]
            vaug = const.tile([P, NKB, HPC, HD + 1], bf16, name="vaug")

            # zero the dead rows of qpad (other head's slot in the pair)
            for h in range(HPC):
                dead = qpad[h][HD:, :] if h % 2 == 0 else qpad[h][:HD, :]
                nc.vector.memset(dead.bitcast(f32), 0.0)
            # ones row for the 1/denominator broadcast matmul (f32r so
            # the 512-row matmul runs at 1 cycle/row)
            ones1r = const.tile([1, HD], f32r, name="ones1r")
            nc.vector.memset(ones1r[:].bitcast(f32), 1.0)
            # ones column of V_aug via f32 scratch -> bf16 strided copy
            onescr = small_pool.tile(
                [P, NKB, HPC, 1], f32, tag="ones", name="ones", bufs=1
            )
            nc.vector.memset(onescr[:], 1.0)
            nc.vector.tensor_copy(vaug[:, :, :, HD : HD + 1], onescr[:])

            pools = (xt_pool, pt_pool, atn_pool, out_pool, small_pool,
                     ps_s, ps_b, ps_av)
            consts = (wqkv_sb, wp_sb, bqk_sb, maskones, ones1r, qpad,
                      ktb, vaug, xt_d, out_d)

            # ---- pipeline with fine-grained weaving ----
            # Attention is Act-engine (exp) bound locally, so between every
            # attention sub-burst we weave dependency-free PE filler units
            # (next chunk's QKV, previous chunk's proj) to keep the PE busy
            # while the Act engine works through the exp backlog.
            filler = _qkv_units(nc, pools, consts, 0, xt=xt0)
            for u in filler:
                u()
            prev_proj = None
            for c in range(NCH):
                filler = []
                if c + 1 < NCH:
                    filler += _qkv_units(nc, pools, consts, c + 1)
                if prev_proj is not None:
                    filler += _proj_units(nc, pools, consts, *prev_proj)
                fi = iter(filler)

                def fill(n):
                    for _ in range(n):
                        u = next(fi, None)
                        if u is not None:
                            u()

                atn_pair = [
                    atn_pool.tile([P, TCH], bf16, tag=f"atn{p}", name=f"atn{p}")
                    for p in range(2)
                ]
                pts = [{} for _ in range(HPC)]
                _emit_scores(nc, pools, consts, 0, c, pts[0])
                fill(1)
                _emit_scores(nc, pools, consts, 1, c, pts[1])
                fill(1)
                _emit_av_norm(nc, pools, consts, 0, c, pts[0], atn_pair)
                fill(1)
                _emit_scores(nc, pools, consts, 2, c, pts[2])
                fill(1)
                _emit_av_norm(nc, pools, consts, 1, c, pts[1], atn_pair)
                fill(1)
                _emit_scores(nc, pools, consts, 3, c, pts[3])
                fill(1)
                _emit_av_norm(nc, pools, consts, 2, c, pts[2], atn_pair)
                fill(2)
                _emit_av_norm(nc, pools, consts, 3, c, pts[3], atn_pair)
                fill(len(filler))
                prev_proj = (c, atn_pair)
            for u in _proj_units(nc, pools, consts, *prev_proj):
                u()

    nc.compile()
    return nc


_CACHE = {}


def get_program():
    if "nc" not in _CACHE:
        _CACHE["nc"] = _build_program()
    return _CACHE["nc"]


def make_in_maps(hidden_states, c_attn_w, c_attn_b, c_proj_w):
    import ml_dtypes

    bf = ml_dtypes.bfloat16
    x = np.asarray(hidden_states, dtype=np.float32)
    wa = np.asarray(c_attn_w, dtype=np.float32)
    ba = np.asarray(c_attn_b, dtype=np.float32)
    wp = np.asarray(c_proj_w, dtype=np.float32)

    xts = [np.ascontiguousarray(x[b].T).astype(bf) for b in range(B)]
    m = np.tril(np.ones((P, P), dtype=np.float32)).T  # m[k, j] = k <= j
    consts = np.ascontiguousarray(m).astype(bf)

    in_maps = []
    for c in range(N_CORES):
        b, g = divmod(c, GROUPS)
        lo, hi = g * FPC, (g + 1) * FPC
        w_qkv = np.ascontiguousarray(
            np.concatenate(
                [wa[:, lo:hi], wa[:, D + lo : D + hi], wa[:, 2 * D + lo : 2 * D + hi]],
                axis=1,
            )
        ).astype(bf)
        b_qk = np.ascontiguousarray(
            np.concatenate([ba[lo:hi], ba[D + lo : D + hi]])
        ).astype(np.float32)
        w_proj = np.ascontiguousarray(wp[lo:hi, :]).astype(bf)
        in_maps.append({
            "xt": xts[b],
            "w_qkv": w_qkv,
            "b_qk": b_qk,
            "w_proj": w_proj,
            "consts": consts,
        })
    return in_maps


def kernel(hidden_states, c_attn_w, c_attn_b, c_proj_w, c_proj_b):
    nc = get_program()
    in_maps = make_in_maps(hidden_states, c_attn_w, c_attn_b, c_proj_w)
    res = run_bass_kernel_spmd(nc, in_maps, list(range(N_CORES)))
    # unshard: sum the 4 head-group partials per batch; V bias commutes
    # through the attention average, so it folds into the proj bias here.
    ba = np.asarray(c_attn_b, dtype=np.float32)
    bias = np.asarray(c_proj_b, dtype=np.float32) + ba[2 * D :] @ np.asarray(
        c_proj_w, dtype=np.float32
    )
    out = np.empty((B, S, D), dtype=np.float32)
    for b in range(B):
        acc = res.results[4 * b]["out"]
        for g in range(1, GROUPS):
            acc = acc + res.results[4 * b + g]["out"]
        out[b] = acc + bias[None, :]
    return out


if __name__ == "__main__":
    rng = np.random.default_rng(0)
    hs = rng.standard_normal((B, S, D), dtype=np.float32)
    wa = rng.standard_normal((D, 3 * D), dtype=np.float32) * 0.02
    ba = rng.standard_normal((3 * D,), dtype=np.float32) * 0.02
    wp = rng.standard_normal((D, D), dtype=np.float32) * 0.02
    bp = rng.standard_normal((D,), dtype=np.float32) * 0.02
    out = kernel(hs, wa, ba, wp, bp)
    print("out", out.shape, out.dtype, float(np.abs(out).max()))
